# revision 1
# baseline (speedup 1.0000x reference)
"""DepthRelationEmbedding Trainium2 kernel (v3).

Math: out[h,n,hw] = relu( sum_d pos[n,hw,d] * W[d,h] + b[h] ) where pos is the
interleaved sin/cos embedding of delta[n,hw] = ln((relu(pd[n])+eps)/(dm[hw]+eps)).

Angle addition (We = W[0::2], Wo = W[1::2]):
  out[(n,h), hw] = sum_k U[k,(n,h)]*cosC[k,hw] + V[k,(n,h)]*sinC[k,hw]
  U = sinA*We + cosA*Wo,  V = sinA*Wo - cosA*We
so the (N,HW,256) intermediate never exists; per core the output is one
[608 x 256] @ [256 x 960] bf16 matmul pair accumulated in PSUM.

Angles in turns (tau = angle/2pi); range reduction f = tau - rint(tau) via the
fp32->int32 copy (rounds-to-nearest on HW); sin = Sin(2pi f),
cos = Sin(pi/2 - 2pi|f|). dm arrives host-reshaped as a [1, 960] row so the
tau_C outer product needs no on-device flatten; Ln runs on ACT (natural_log
table), trig on ACT (trig table) - exactly two table loads, both early.
Everything downstream of the trig (U/V build, cs rows, main matmuls, output)
is bf16; the output DMA moves half the bytes and the host upcasts.

Sharding: SN x SH = 4 x 2 cores over (N, HW).
"""

import sys

for p in ("/opt/trn_rl_repo", "/root/.axon_site/_ro/trn_rl_repo"):
    if p not in sys.path:
        sys.path.insert(0, p)

import numpy as np
import ml_dtypes
from contextlib import ExitStack

from concourse import bacc, mybir, tile
from concourse.bass_utils import run_bass_kernel_spmd

F32 = mybir.dt.float32
BF16 = mybir.dt.bfloat16
I32 = mybir.dt.int32
A = mybir.AluOpType
AF = mybir.ActivationFunctionType

# ---- problem constants ----
N_TOT, H_DM, W_DM = 300, 24, 80
HW_TOT = H_DM * W_DM  # 1920
HEADS = 8
ED = 256
K = ED // 2  # 128
EPS = 1e-5
SCALE = 100.0
TEMPERATURE = 10000.0
TWO_PI = 2.0 * np.pi

# ---- sharding ----
SN, SH = 4, 2
n_per = N_TOT // SN  # 75
n_pad = 76
hw_per = HW_TOT // SH  # 960
M = n_pad * HEADS  # 608
HM = M // 2  # 304
CH = 480  # hw chunk (1 psum bank)

_m_tiles = []
_ms = 0
while _ms < M:
    _m_tiles.append((_ms, min(128, M - _ms)))
    _ms += 128


def _sigma_row():
    k = np.arange(K, dtype=np.float64)
    dim_t = np.float64(TEMPERATURE) ** (k * 2.0 / ED)
    return ((SCALE / dim_t) / TWO_PI).astype(np.float32)[None, :]  # [1,128] turns


def _build_program():
    nc = bacc.Bacc("TRN2", target_bir_lowering=False, debug=False)

    pd_d = nc.dram_tensor("pdrow", [1, n_pad], F32, kind="ExternalInput")
    dm_d = nc.dram_tensor("dmrow", [2, CH], F32, kind="ExternalInput")
    sig_d = nc.dram_tensor("sig", [1, K], F32, kind="ExternalInput")
    sig2a_d = nc.dram_tensor("sig2a", [2, K], F32, kind="ExternalInput")
    sig2b_d = nc.dram_tensor("sig2b", [2, K], F32, kind="ExternalInput")
    wew_d = nc.dram_tensor("wew", [K, 2 * HEADS], BF16, kind="ExternalInput")
    wow_d = nc.dram_tensor("wow", [K, 2 * HEADS], BF16, kind="ExternalInput")
    bias_d = nc.dram_tensor("bias_rep", [128, 1], F32, kind="ExternalInput")
    wa_d = nc.dram_tensor("warm_a", [128, 128], BF16, kind="ExternalInput")
    wb_d = nc.dram_tensor("warm_b", [128, CH], BF16, kind="ExternalInput")
    out_d = nc.dram_tensor("out", [M, hw_per], BF16, kind="ExternalOutput")

    with tile.TileContext(nc) as tc, ExitStack() as ctx:
        sb = ctx.enter_context(tc.tile_pool(name="sb", bufs=1))
        pso = ctx.enter_context(tc.tile_pool(name="pso", bufs=5, space="PSUM"))
        psm = ctx.enter_context(tc.tile_pool(name="psm", bufs=3, space="PSUM"))

        # eps on gpsimd (earliest engine) so the Ln warm can issue ~6.3us
        eps_c = sb.tile((128, 1), F32, tag="c_eps")
        nc.gpsimd.memset(eps_c[:], EPS)
        twopi_c = sb.tile((128, 1), F32, tag="c_2pi")
        nc.vector.memset(twopi_c[:], TWO_PI)
        negtwopi_c = sb.tile((128, 1), F32, tag="c_n2pi")
        nc.vector.memset(negtwopi_c[:], -TWO_PI)
        halfpi_c = sb.tile((128, 1), F32, tag="c_hpi")
        nc.vector.memset(halfpi_c[:], np.pi / 2)

        # natural_log table load triggers here, before any data arrives
        lnw = sb.tile((1, 1), F32, tag="lnw")
        nc.scalar.activation(lnw[:], eps_c[0:1], AF.Ln, bias=eps_c[0:1])

        # ---- input DMAs ----
        pdr = sb.tile((1, n_pad), F32, tag="pdr")
        nc.sync.dma_start(pdr[:], pd_d[:])
        dmr = sb.tile((2, CH), F32, tag="dmr")
        nc.sync.dma_start(dmr[:], dm_d[:])

        wa = sb.tile((128, 128), BF16, tag="wa")
        nc.sync.dma_start(wa[:], wa_d[:])
        wb = sb.tile((128, CH), BF16, tag="wb")
        nc.sync.dma_start(wb[:], wb_d[:])
        lhs_s = sb.tile((1, K), F32, tag="lhs_s")
        nc.gpsimd.dma_start(lhs_s[:], sig_d[:])
        lhs_s2a = sb.tile((2, K), F32, tag="lhs_s2a")
        nc.gpsimd.dma_start(lhs_s2a[:], sig2a_d[:])
        lhs_s2b = sb.tile((2, K), F32, tag="lhs_s2b")
        nc.gpsimd.dma_start(lhs_s2b[:], sig2b_d[:])
        bias_t = sb.tile((128, 1), F32, tag="bias")
        nc.gpsimd.dma_start(bias_t[:], bias_d[:])
        wew_t = sb.tile((K, 2 * HEADS), BF16, tag="wew")
        nc.gpsimd.dma_start(wew_t[:], wew_d[:])
        wow_t = sb.tile((K, 2 * HEADS), BF16, tag="wow")
        nc.gpsimd.dma_start(wow_t[:], wow_d[:])



        # ---- logs on ACT: rhs rows for the tau outer products ----
        # dm first (gates the big tau_C matmuls), then pd
        rhs_c = sb.tile((2, CH), F32, tag="rhs_c")
        nc.scalar.activation(rhs_c[:], dmr[:], AF.Ln, bias=eps_c[0:2])
        nc.vector.tensor_scalar(pdr[:], pdr[:], 0.0, None, A.max)
        rhs_a = sb.tile((1, n_pad), F32, tag="rhs_a")
        nc.scalar.activation(rhs_a[:], pdr[:], AF.Ln, bias=eps_c[0:1])

        # ---- PE: warmups (HAM ramp) interleaved with small fp32 tau mms ----
        ps_w = pso.tile((128, CH), F32, tag="pso")
        for _ in range(3):
            nc.tensor.matmul(ps_w[:], wa[:], wb[:], start=True, stop=True)
        ps_a = psm.tile((K, CH), F32, tag="psm")
        nc.tensor.matmul(ps_a[:, 0:n_pad], lhs_s[:], rhs_a[:], start=True, stop=True)
        ps_c = []
        for ci in range(2):
            p = psm.tile((K, CH), F32, tag="psm")
            lhs2 = lhs_s2a if ci == 0 else lhs_s2b
            nc.tensor.matmul(p[:], lhs2[:], rhs_c[:], start=True, stop=True)
            ps_c.append(p)
        for _ in range(3):
            nc.tensor.matmul(ps_w[:], wa[:], wb[:], start=True, stop=True)

        # ---- range reduction + trig ----
        trigA = sb.tile((K, 2 * n_pad), BF16, tag="trigA")  # [sinA | cosA]
        qA = sb.tile((K, n_pad), I32, tag="qA")
        fA = sb.tile((K, n_pad), F32, tag="fA")
        uA = sb.tile((K, n_pad), F32, tag="uA")
        qC = sb.tile((K, hw_per), I32, tag="qC")
        fC = sb.tile((K, hw_per), F32, tag="fC")
        uC = sb.tile((K, hw_per), F32, tag="uC")
        cs_sin = sb.tile((K, hw_per), BF16, tag="cs_sin")
        cs_cos = sb.tile((K, hw_per), BF16, tag="cs_cos")

        def gsl(ci):
            return slice(ci * CH, (ci + 1) * CH)

        # A-side reduce on DVE (tiny)
        nc.vector.tensor_copy(qA[:], ps_a[:, 0:n_pad])
        nc.vector.tensor_tensor(fA[:], ps_a[:, 0:n_pad], qA[:], A.subtract)
        nc.vector.tensor_scalar(
            uA[:].bitcast(I32), fA[:].bitcast(I32), 0x7FFFFFFF, None, A.bitwise_and
        )

        def redC(ci):  # rint on ACT (Copy), sub + abs on DVE
            sl = gsl(ci)
            nc.vector.tensor_copy(qC[:, sl], ps_c[ci][:])
            nc.vector.tensor_tensor(fC[:, sl], ps_c[ci][:], qC[:, sl], A.subtract)
            nc.vector.tensor_scalar(
                uC[:, sl].bitcast(I32), fC[:, sl].bitcast(I32),
                0x7FFFFFFF, None, A.bitwise_and,
            )

        def sin2pi(out_ap, in_ap):
            nc.scalar.activation(out_ap, in_ap, AF.Sin, scale=twopi_c[:])

        def cos2pi(out_ap, in_ap):  # in = |f|
            nc.scalar.activation(
                out_ap, in_ap, AF.Sin, bias=halfpi_c[:], scale=negtwopi_c[:]
            )

        # ---- U/V build on DVE, all bf16; weights pre-expanded on host so
        # every operand is packed (4x DVE mode, no stride-0 inner) ----
        U = sb.tile((K, M), BF16, tag="U")
        V = sb.tile((K, M), BF16, tag="V")
        tmp1 = sb.tile((K, 2 * M), BF16, tag="tmp1")
        tmp2 = sb.tile((K, 2 * M), BF16, tag="tmp2")

        def trig_bc():  # [K, 2*n_pad] -> [K, 2, n_pad, HEADS]
            return (
                trigA[:]
                .rearrange("p (s n) -> p s n", s=2)
                .unsqueeze(3)
                .to_broadcast((K, 2, n_pad, HEADS))
            )

        def r4(t):
            return t[:].rearrange("p (s n h) -> p s n h", s=2, h=HEADS)

        def w_bc(t):  # [K, 2*HEADS] -> [K, 2, n_pad, HEADS]
            return (
                t[:]
                .rearrange("p (s h) -> p s h", s=2)
                .unsqueeze(2)
                .to_broadcast((K, 2, n_pad, HEADS))
            )

        def build(out_t, tmp_t, w_t, op):
            nc.vector.tensor_tensor(r4(tmp_t), trig_bc(), w_bc(w_t), A.mult)
            nc.vector.tensor_tensor(
                out_t[:, 0:HM], tmp_t[:, 0:HM], tmp_t[:, M : M + HM], op
            )
            nc.vector.tensor_tensor(
                out_t[:, HM:M], tmp_t[:, HM:M], tmp_t[:, M + HM : 2 * M], op
            )

        # ACT order: sinA, cosA (unblock U), qC0, cosC0/sinC0, qC1, cosC1/sinC1
        sin2pi(trigA[:, 0:n_pad], fA[:])
        cos2pi(trigA[:, n_pad:], uA[:])
        build(U, tmp1, wew_t, A.add)
        redC(0)
        sin2pi(cs_sin[:, gsl(0)], fC[:, gsl(0)])
        cos2pi(cs_cos[:, gsl(0)], uC[:, gsl(0)])
        build(V, tmp2, wow_t, A.subtract)
        redC(1)
        sin2pi(cs_sin[:, gsl(1)], fC[:, gsl(1)])
        cos2pi(cs_cos[:, gsl(1)], uC[:, gsl(1)])

        # ---- main matmuls + bias/relu + store ----
        obs = {}
        for ci in range(2):
            sl = gsl(ci)
            ps_os = []
            for mi, (ms, mr) in enumerate(_m_tiles):
                ps_o = pso.tile((128, CH), F32, tag="pso")
                ps_os.append(ps_o)
                nc.tensor.matmul(
                    ps_o[:mr, :], U[:, ms : ms + mr], cs_cos[:, sl],
                    start=True, stop=False,
                )
            for mi, (ms, mr) in enumerate(_m_tiles):
                ps_o = ps_os[mi]
                nc.tensor.matmul(
                    ps_o[:mr, :], V[:, ms : ms + mr], cs_sin[:, sl],
                    start=False, stop=True,
                )
                if ci == 0:
                    ob_new = sb.tile((128, hw_per), BF16, tag=f"ob{mi}")
                    obs[mi] = ob_new
                ob = obs[mi]
                ti = ci * len(_m_tiles) + mi
                if ti % 2 == 0:
                    nc.scalar.activation(
                        ob[:mr, sl], ps_o[:mr, :], AF.Relu, bias=bias_t[0:mr]
                    )
                else:
                    nc.vector.tensor_scalar(
                        ob[:mr, sl], ps_o[:mr, :], bias_t[0:mr], 0.0, A.add, A.max
                    )
                if ci == 1:
                    nc.sync.dma_start(out_d[ms : ms + mr, :], ob[:mr, :])

    nc.finalize()
    return nc


_NC = None


def _get_nc():
    global _NC
    if _NC is None:
        _NC = _build_program()
    return _NC


def _make_in_maps(predict_depth, depth_map, W, b):
    pd = np.asarray(predict_depth, np.float32).reshape(N_TOT)
    dm = np.asarray(depth_map, np.float32).reshape(HW_TOT)
    W = np.asarray(W, np.float32)
    b = np.asarray(b, np.float32)

    we = W[0::2, :]
    wo = W[1::2, :]
    wew = np.stack([we, wo], axis=1).reshape(K, 2 * HEADS).astype(ml_dtypes.bfloat16)
    wow = np.stack([wo, we], axis=1).reshape(K, 2 * HEADS).astype(ml_dtypes.bfloat16)
    bias_rep = np.ascontiguousarray(np.tile(b, 16)[:, None])
    sig = np.ascontiguousarray(_sigma_row())
    sig2a = np.zeros((2, K), np.float32)
    sig2a[0] = sig[0]
    sig2b = np.zeros((2, K), np.float32)
    sig2b[1] = sig[0]

    rng = np.random.default_rng(0)
    wa = rng.standard_normal((128, 128), np.float32).astype(ml_dtypes.bfloat16)
    wb = rng.standard_normal((128, CH), np.float32).astype(ml_dtypes.bfloat16)

    in_maps = []
    for c in range(SN * SH):
        ni, hi = c // SH, c % SH
        pd_row = np.zeros((1, n_pad), np.float32)
        pd_row[0, :n_per] = pd[ni * n_per : (ni + 1) * n_per]
        dm_row = np.ascontiguousarray(
            dm[hi * hw_per : (hi + 1) * hw_per].reshape(2, CH)
        )
        in_maps.append(
            {
                "pdrow": pd_row,
                "dmrow": dm_row,
                "sig": sig,
                "sig2a": sig2a,
                "sig2b": sig2b,
                "wew": np.ascontiguousarray(wew),
                "wow": np.ascontiguousarray(wow),
                "bias_rep": bias_rep,
                "warm_a": wa,
                "warm_b": wb,
            }
        )
    return in_maps


def _run(inputs, trace=False):
    nc = _get_nc()
    in_maps = _make_in_maps(**inputs)
    res = run_bass_kernel_spmd(nc, in_maps, core_ids=list(range(SN * SH)), trace=trace)
    out = np.empty((HEADS, N_TOT, HW_TOT), np.float32)
    for c in range(SN * SH):
        ni, hi = c // SH, c % SH
        blk = (
            np.asarray(res.results[c]["out"])
            .astype(np.float32)
            .reshape(n_pad, HEADS, hw_per)
            .transpose(1, 0, 2)
        )
        n0 = ni * n_per
        out[:, n0 : n0 + n_per, hi * hw_per : (hi + 1) * hw_per] = blk[:, :n_per, :]
    return out, res


def kernel(predict_depth, depth_map, W, b):
    out, _ = _run(
        {"predict_depth": predict_depth, "depth_map": depth_map, "W": W, "b": b}
    )
    return out

